# revision 14
# baseline (speedup 1.0000x reference)
"""Trainium2 Bass kernel for per-sample modulated 3x3 conv (StyleGAN2-style).

Math: the reference modulates the shared conv weight per (batch, out-channel)
and demodulates by the weight norm.  Since the modulation is a pure
per-(b,o) scale, the grouped conv factorizes:

    out[b,o] = m[b,o] * conv2d(x[b], W) + conv_bias[o]
    m[b,o]   = eq_c * s[b,o] * rsqrt(s[b,o]^2 * eq_c^2 * q[o] + 1e-8)
    q[o]     = sum_{i,kh,kw} W[o,i,kh,kw]^2
    s[b,o]   = style_w[b] @ (lin_w[o] * eq_l) + lin_b[o] + 1 + mod_bias[o]

so no per-sample weights are ever materialized.  Sharding: data-parallel,
2 samples per core across 8 cores; weights/style params replicated.

Per-core kernel: conv as 576 bf16 matmuls (4 o-tiles x 4 i-tiles x 9 taps
x 2 samples x 2 hw-tiles of [128c x 512hw]) accumulated in PSUM over
(i-tile, tap).  Weights are transposed [o,i] -> [i,o] on the tensor engine
via identity matmuls (bf16), three taps packed per PSUM bank with one
batched vector copy.  The style path runs in fp32 on the tensor engine
(sign of s matters near zero: min |s| ~ 8e-4 and demodulation saturates,
so bf16 there would flip output signs), q and the final scale in fp32 on
the vector/scalar engines.  `stage` and `loop_n` are self-test hooks:
kernel() always builds stage="full", loop_n=0.
"""

import numpy as np
import ml_dtypes
from math import sqrt

B, C, O, K, LAT, H, W = 16, 512, 512, 3, 256, 32, 32
N_CORES = 8
S = B // N_CORES          # samples per core
P = 128                   # partitions
OT = O // P               # o-tiles
IT = C // P               # i-tiles
KK = K * K                # taps
HP = H + 2                # padded spatial
HT = 2                    # hw row-tiles (16 rows x 32 cols = 512 = one PSUM bank)
HR = H // HT              # rows per hw tile
EQ_CONV = sqrt(2.0 / (C * K * K))
EQ_LIN = sqrt(2.0 / LAT)

_CACHE = {}


def _build(stage="full", loop_n=0):
    import contextlib
    import concourse.bacc as bacc
    import concourse.tile as tile
    from concourse import mybir

    f32 = mybir.dt.float32
    bf16 = mybir.dt.bfloat16
    MUL = mybir.AluOpType.mult
    ADD = mybir.AluOpType.add

    nc = bacc.Bacc("TRN2", target_bir_lowering=False, debug=False)

    x_d = nc.dram_tensor("x", [S, C, H, W], f32, kind="ExternalInput").ap()
    sw_d = nc.dram_tensor("style_w", [S, LAT], f32, kind="ExternalInput").ap()
    cw_d = nc.dram_tensor("conv_weight", [O, C, K, K], f32, kind="ExternalInput").ap()
    lw_d = nc.dram_tensor("lin_w", [O, LAT], f32, kind="ExternalInput").ap()
    lb_d = nc.dram_tensor("lin_b", [O], f32, kind="ExternalInput").ap()
    mb_d = nc.dram_tensor("mod_bias", [O], f32, kind="ExternalInput").ap()
    cb_d = nc.dram_tensor("conv_bias", [O], f32, kind="ExternalInput").ap()
    eyeb_d = nc.dram_tensor("eye_bf", [P, P], bf16, kind="ExternalInput").ap()
    eyef_d = nc.dram_tensor("eye_f32", [P, P], f32, kind="ExternalInput").ap()
    eye2_d = nc.dram_tensor("eye2", [S, S], f32, kind="ExternalInput").ap()
    out_d = nc.dram_tensor("out", [S, O, H, W], f32, kind="ExternalOutput").ap()

    cw2 = cw_d.rearrange("o i kh kw -> o (i kh kw)")          # [512, 4608]
    lb2 = lb_d.rearrange("(t p u) -> t p u", p=P, u=1)        # [4, 128, 1]
    mb2 = mb_d.rearrange("(t p u) -> t p u", p=P, u=1)
    cb2 = cb_d.rearrange("(t p u) -> t p u", p=P, u=1)

    with tile.TileContext(nc) as tc:
        loop = tc.For_i(0, loop_n, 1) if loop_n else contextlib.nullcontext()
        with (
            loop,
            tc.tile_pool(name="const", bufs=1) as cp,
            tc.tile_pool(name="work", bufs=2) as wp,
            tc.tile_pool(name="outp", bufs=4) as op_,
            tc.tile_pool(name="cpsum", bufs=6, space="PSUM") as pconv,
            tc.tile_pool(name="tpsum", bufs=2, space="PSUM") as ptp,
        ):
            # ---------- input DMAs ----------
            eyeb = cp.tile([P, P], bf16, tag="eyeb")
            nc.sync.dma_start(eyeb[:], eyeb_d[:])
            eyef = cp.tile([P, P], f32, tag="eyef")
            nc.sync.dma_start(eyef[:], eyef_d[:])
            eye2 = cp.tile([S, S], f32, tag="eye2")
            nc.sync.dma_start(eye2[:], eye2_d[:])

            style_sb = cp.tile([S, LAT], f32, tag="style")
            nc.sync.dma_start(style_sb[:], sw_d[:])
            linw = []
            for ot in range(OT):
                t = cp.tile([P, LAT], f32, tag=f"linw{ot}")
                nc.sync.dma_start(t[:], lw_d[ot * P:(ot + 1) * P, :])
                linw.append(t)
            lbv, mbv, cbv = [], [], []
            for ot in range(OT):
                t = cp.tile([P, 1], f32, tag=f"lb{ot}")
                nc.sync.dma_start(t[:], lb2[ot])
                lbv.append(t)
                t = cp.tile([P, 1], f32, tag=f"mb{ot}")
                nc.sync.dma_start(t[:], mb2[ot])
                mbv.append(t)
                t = cp.tile([P, 1], f32, tag=f"cb{ot}")
                nc.sync.dma_start(t[:], cb2[ot])
                cbv.append(t)

            # natural-layout bf16 weights (cast during SWDGE DMA).  o-tile 0
            # first so its transposes start ASAP; x tiles next (first conv
            # chain needs xpad + wt[0]); remaining weight tiles after.
            wnat = [cp.tile([P, C * KK], bf16, tag=f"wnat{ot}", name=f"wnat{ot}")
                    for ot in range(OT)]
            nc.gpsimd.dma_start(wnat[0][:], cw2[0:P, :])

            # zero-padded bf16 inputs (border fill on the startup-idle DVE)
            xpad = [[None] * IT for _ in range(S)]
            for it in range(IT):
                for s in range(S):
                    t = cp.tile([P, HP, HP], bf16, tag=f"xpad{s}_{it}",
                                name=f"xpad{s}_{it}")
                    nc.vector.memset(t[:], 0.0)
                    nc.gpsimd.dma_start(
                        t[:, 1:H + 1, 1:W + 1], x_d[s, it * P:(it + 1) * P, :, :]
                    )
                    xpad[s][it] = t
            for ot in range(1, OT):
                nc.gpsimd.dma_start(wnat[ot][:], cw2[ot * P:(ot + 1) * P, :])

            lvl = {"dma": 1, "style": 2, "qm": 3, "transpose": 4, "full": 5}[stage]
            if lvl < 5:
                # stub outputs so every stage writes `out`
                for s in range(S):
                    for ot in range(OT):
                        for ht in range(HT):
                            ob = op_.tile([P, HR * W], f32, tag="ob", name="ob")
                            nc.vector.memset(ob[:], 0.0)
                            nc.sync.dma_start(
                                out_d[s, ot * P:(ot + 1) * P,
                                      ht * HR:(ht + 1) * HR, :],
                                ob[:].rearrange("p (h w) -> p h w", w=W),
                            )

            # ---------- style path (fp32 on PE; sign of s matters) ----------
            # styleT[lt] = style_w[:, lt*128:+128].T  -> [128 lat, S]
            styleT = []
            for lt in range(LAT // P) if lvl >= 2 else []:
                ps = ptp.tile([P, S], f32, tag="tp")
                nc.tensor.matmul(ps[:], style_sb[:, lt * P:(lt + 1) * P], eye2[:],
                                 start=True, stop=True)
                t = cp.tile([P, S], f32, tag=f"styleT{lt}")
                nc.vector.tensor_copy(t[:], ps[:])
                styleT.append(t)
            # linwT[lt][ot] = lin_w[ot-block, lt-block].T -> [128 lat, 128 o]
            linwT = [[None] * OT for _ in range(LAT // P)]
            for ot in range(OT) if lvl >= 2 else []:
                for lt in range(LAT // P):
                    ps = ptp.tile([P, P], f32, tag="tp")
                    nc.tensor.matmul(ps[:], linw[ot][:, lt * P:(lt + 1) * P],
                                     eyef[:], start=True, stop=True)
                    t = cp.tile([P, P], f32, tag=f"linwT{lt}_{ot}")
                    nc.vector.tensor_copy(t[:], ps[:])
                    linwT[lt][ot] = t
            # s_mm[ot] = sum_lt linwT[lt][ot].T @ styleT[lt] -> [128 o, S]
            su = []
            for ot in range(OT) if lvl >= 2 else []:
                ps = ptp.tile([P, S], f32, tag="tp")
                for lt in range(LAT // P):
                    nc.tensor.matmul(ps[:], linwT[lt][ot][:], styleT[lt][:],
                                     start=(lt == 0), stop=(lt == LAT // P - 1))
                # bvec = lin_b + mod_bias + 1
                bv = wp.tile([P, 1], f32, tag="bv")
                nc.vector.tensor_add(bv[:], lbv[ot][:], mbv[ot][:])
                bv2 = wp.tile([P, 1], f32, tag="bv2")
                nc.vector.tensor_scalar_add(bv2[:], bv[:], 1.0)
                # su = s_mm * eq_lin + bvec
                t = cp.tile([P, S], f32, tag=f"su{ot}")
                nc.vector.tensor_scalar(t[:], ps[:], EQ_LIN, bv2[:], MUL, ADD)
                su.append(t)

            # ---------- per-o-tile: q, m, weight transpose, conv ----------
            wt = [None] * OT  # wt[ot]: [128 i_local, (kk, it, o)]
            mcol = []
            for ot in range(OT) if lvl >= 3 else []:
                # q[o] = sum_{i,kk} w_bf16^2   (fp32 accum, matches PE weights;
                # ACT-engine Square with free-axis accumulate)
                qscr = wp.tile([P, C * KK], bf16, tag="qscr")
                qv = cp.tile([P, 1], f32, tag=f"q{ot}")
                nc.scalar.activation(qscr[:], wnat[ot][:],
                                     mybir.ActivationFunctionType.Square,
                                     accum_out=qv[:])
                # m = u * rsqrt(u^2 * q + 1e-8),  u = su * eq_conv
                uv = wp.tile([P, S], f32, tag="uv")
                nc.vector.tensor_scalar_mul(uv[:], su[ot][:], EQ_CONV)
                u2 = wp.tile([P, S], f32, tag="u2")
                nc.vector.tensor_mul(u2[:], uv[:], uv[:])
                vv = wp.tile([P, S], f32, tag="vv")
                nc.vector.tensor_scalar(vv[:], u2[:], qv[:], 1e-8, MUL, ADD)
                sq = wp.tile([P, S], f32, tag="sq")
                nc.scalar.sqrt(sq[:], vv[:])
                rv = wp.tile([P, S], f32, tag="rv")
                nc.vector.reciprocal(rv[:], sq[:])
                mv = cp.tile([P, S], f32, tag=f"m{ot}")
                nc.vector.tensor_mul(mv[:], uv[:], rv[:])
                mcol.append(mv)

                # transpose W[ot-block] -> wt[ot] via identity matmuls;
                # 4 taps share one PSUM bank, one batched DVE copy per group
                wn_r = wnat[ot][:].rearrange("p (i k) -> p i k", k=KK)
                if lvl >= 4:
                    t = cp.tile([P, KK * IT * P], bf16, tag=f"wt{ot}")
                    wt[ot] = t
                    t4 = t[:].rearrange("p (k x o) -> p k x o", k=KK, x=IT)
                    for it in range(IT):
                        for g in range(3):           # groups of 3 taps share a bank
                            ps = ptp.tile([P, 3 * P], f32, tag="tp", name="tp")
                            for j in range(3):
                                kk = g * 3 + j
                                nc.tensor.matmul(ps[:, j * P:(j + 1) * P],
                                                 wn_r[:, it * P:(it + 1) * P, kk],
                                                 eyeb[:], start=True, stop=True)
                            nc.vector.tensor_copy(
                                t4[:, g * 3:(g + 1) * 3, it, :],
                                ps[:].rearrange("p (k o) -> p k o", k=3))

                # conv: accumulate over (it, kk) into PSUM per (s, ht)
                if lvl < 5:
                    continue
                cps = [[pconv.tile([P, HR * W], f32, tag="cps", name="cps")
                        for _ in range(HT)] for _ in range(S)]
                wt4 = wt[ot][:].rearrange("p (k x o) -> p k x o", k=KK, x=IT)
                for it in range(IT):
                    for kk in range(KK):
                        dh, dw = kk // K, kk % K
                        lhs = wt4[:, kk, it, :]
                        first = (it == 0 and kk == 0)
                        last = (it == IT - 1 and kk == KK - 1)
                        for s in range(S):
                            for ht in range(HT):
                                rhs = xpad[s][it][:, ht * HR + dh:ht * HR + dh + HR,
                                                  dw:dw + W]
                                nc.tensor.matmul(cps[s][ht][:], lhs, rhs,
                                                 start=first, stop=last)
                # epilogue: out = psum * m + conv_bias, store
                for s in range(S):
                    for ht in range(HT):
                        ob = op_.tile([P, HR * W], f32, tag="ob")
                        nc.vector.tensor_scalar(
                            ob[:], cps[s][ht][:], mcol[ot][:, s:s + 1],
                            cbv[ot][:], MUL, ADD,
                        )
                        nc.sync.dma_start(
                            out_d[s, ot * P:(ot + 1) * P,
                                  ht * HR:(ht + 1) * HR, :],
                            ob[:].rearrange("p (h w) -> p h w", w=W),
                        )

    nc.compile()
    return nc


def kernel(x, style_w, conv_weight, lin_w, lin_b, mod_bias, conv_bias):
    from concourse.bass_utils import run_bass_kernel_spmd

    if "nc" not in _CACHE:
        _CACHE["nc"] = _build()
    nc = _CACHE["nc"]

    eye_bf = np.eye(P, dtype=ml_dtypes.bfloat16)
    eye_f32 = np.eye(P, dtype=np.float32)
    eye2 = np.eye(S, dtype=np.float32)
    x = np.ascontiguousarray(np.asarray(x, dtype=np.float32))
    style_w = np.ascontiguousarray(np.asarray(style_w, dtype=np.float32))
    conv_weight = np.ascontiguousarray(np.asarray(conv_weight, dtype=np.float32))
    lin_w = np.ascontiguousarray(np.asarray(lin_w, dtype=np.float32))
    lin_b = np.ascontiguousarray(np.asarray(lin_b, dtype=np.float32))
    mod_bias = np.ascontiguousarray(np.asarray(mod_bias, dtype=np.float32))
    conv_bias = np.ascontiguousarray(np.asarray(conv_bias, dtype=np.float32))

    in_maps = []
    for c in range(N_CORES):
        in_maps.append({
            "x": x[c * S:(c + 1) * S],
            "style_w": style_w[c * S:(c + 1) * S],
            "conv_weight": conv_weight,
            "lin_w": lin_w,
            "lin_b": lin_b,
            "mod_bias": mod_bias,
            "conv_bias": conv_bias,
            "eye_bf": eye_bf,
            "eye_f32": eye_f32,
            "eye2": eye2,
        })
    res = run_bass_kernel_spmd(nc, in_maps, list(range(N_CORES)))
    return np.concatenate([res.results[c]["out"] for c in range(N_CORES)], axis=0)
